# revision 21
# baseline (speedup 1.0000x reference)
"""Trainium2 Bass kernel for the GWNN2 GNN (4-graph GraphConv x2 + MLP).

v3 strategy (8 NeuronCores, dst-sharded):
  * all deg_in/deg_out normalization folded into per-edge one-hot weights
    (host side).
  * layer-1 conv aggregates FIRST in the 256-d input space directly from the
    replicated x table (512B gather rows, no projection table, no phase 1,
    no first collective), then applies W1 per dst window:
        h1 = relu((A^T x_w) @ W1)
  * layer-2 projection table T2 = h @ W2  [50176, 128] bf16 shared by all
    graphs (sharded compute + small AllGather).
  * SpMM per dst window of 128 nodes: dma_gather (<=1024 idxs per call, HW
    descriptor-ring limit) + one-hot selection built on DVE in 2x mode
    (st layout [P, WIN, KB] keeps every operand's last dim stride-1),
    PSUM-accumulated matmul.
  * dense l1/l2 + W2 fused per window between the two conv layers.
"""
import sys
import types
from dataclasses import dataclass

if "/opt/trn_rl_repo" not in sys.path:
    sys.path.insert(0, "/opt/trn_rl_repo")

import numpy as np
import ml_dtypes

import concourse.bass as bass
import concourse.bacc as bacc
import concourse.mybir as mybir
import concourse.tile as tile
from concourse.masks import make_identity

BF16 = ml_dtypes.bfloat16
P = 128
GCH = 8          # chunks (of 128 idx) per dma_gather call (HW ring limit)


def _install_ntff_hook():
    """Make trace=True usable under axon (antenv.axon_hooks may be absent)."""
    try:
        import antenv
        if "antenv.axon_hooks" in sys.modules:
            return
        m = types.ModuleType("antenv.axon_hooks")
        box = [None]
        m.set_axon_ntff_profile_hook = lambda h: box.__setitem__(0, h)
        m.get_axon_ntff_profile_hook = lambda: box[0]
        sys.modules["antenv.axon_hooks"] = m
        antenv.axon_hooks = m
        try:
            from trn_agent_boot.trn_boot import _ntff_profile_via_ctypes
            hook = _ntff_profile_via_ctypes("/opt/axon/libaxon_pjrt.so")
            if hook is not None:
                m.set_axon_ntff_profile_hook(hook)
        except Exception:
            pass
    except Exception:
        pass


@dataclass
class Cfg:
    n_nodes: int = 50000
    g_num: int = 4
    in_feats: int = 256
    h_feats: int = 128          # table row width (must be 128)
    n_classes: int = 40
    n_cores: int = 8
    win: int = 128              # dst nodes per SpMM window
    wb1: int = 3                # windows per batch, layer-1 (256-wide ft)
    wb2: int = 7                # windows per batch, layer-2

    @property
    def shard(self):
        return self.n_nodes // self.n_cores

    @property
    def shard_p(self):          # padded shard rows
        return ((self.shard + P - 1) // P) * P

    @property
    def rows(self):             # padded table rows
        return self.shard_p * self.n_cores

    @property
    def half(self):
        return self.rows // 2

    @property
    def nwin(self):
        return self.shard_p // self.win

    @property
    def cat(self):
        return self.h_feats * self.g_num

    @property
    def kc_cat(self):           # 128-chunks in cat dim
        return self.cat // P

    @property
    def kc_in(self):
        return self.in_feats // P

    @property
    def ntile(self):            # node tiles (128) in full padded table
        return self.rows // P


def _prep_inputs(cfg: Cfg, in_feat, src, dst, w, W1, W2, l1w, l1b, l2w, l2b,
                 l3w, l3b):
    """Host-side sharding/packing. Returns (in_maps, K_LO, K_HI)."""
    N, G = cfg.n_nodes, cfg.g_num
    SH, SHP = cfg.shard, cfg.shard_p
    NW, WIN = cfg.nwin, cfg.win
    HALF = cfg.half
    src = np.asarray(src).astype(np.int64)
    dst = np.asarray(dst).astype(np.int64)
    w = np.asarray(w, dtype=np.float32)
    in_feat = np.asarray(in_feat, dtype=np.float32)

    deg_out = np.empty((G, N), np.float32)
    deg_in = np.empty((G, N), np.float32)
    for g in range(G):
        deg_out[g] = np.clip(np.bincount(src[g], minlength=N), 1.0, None) ** -0.5
        deg_in[g] = np.clip(np.bincount(dst[g], minlength=N), 1.0, None) ** -0.5

    src_pad = (src // SH) * SHP + (src % SH)          # padded table row
    half_flag = (src_pad >= HALF).astype(np.int64)
    idx_local = (src_pad - half_flag * HALF).astype(np.int64)

    core_of = dst // SH
    dst_loc = dst % SH
    win_of = dst_loc // WIN
    dst_in_win = (dst_loc % WIN).astype(np.float32)

    # first pass: counts to fix K_LO / K_HI globally (one SPMD NEFF)
    maxlo = maxhi = 1
    buckets = {}
    for i in range(cfg.n_cores):
        for g in range(G):
            m = core_of[g] == i
            key = win_of[g][m] * 2 + half_flag[g][m]
            cnt = np.bincount(key, minlength=NW * 2)
            maxlo = max(maxlo, int(cnt[0::2].max()))
            maxhi = max(maxhi, int(cnt[1::2].max()))
            buckets[(i, g)] = m
    K_LO = (maxlo + P - 1) // P
    K_HI = (maxhi + P - 1) // P

    # all normalization folded into the per-edge selection weight
    w_eff = np.empty((G, src.shape[1]), np.float32)
    for g in range(G):
        w_eff[g] = w[g] * deg_in[g][dst[g]] * deg_out[g][src[g]]

    # replicated padded x rows (bf16) for the layer-1 direct gather
    xrows = np.zeros((cfg.rows, cfg.in_feats), np.float32)
    for i in range(cfg.n_cores):
        xrows[i * SHP:i * SHP + SH] = in_feat[i * SH:(i + 1) * SH]
    xrows = xrows.astype(BF16)

    def pack_lhsT(W, kc):
        Wr = np.asarray(W, np.float32).reshape(kc, P, -1)   # (kc, k, fout)
        return np.ascontiguousarray(Wr.transpose(1, 0, 2)).reshape(P, -1)

    W1c = pack_lhsT(W1, cfg.kc_in).astype(BF16)
    W2c = pack_lhsT(W2, cfg.kc_cat).astype(BF16)
    l1wc = pack_lhsT(l1w, cfg.kc_cat).astype(BF16)
    l2wc = pack_lhsT(l2w, cfg.kc_cat).astype(BF16)
    l3wc = pack_lhsT(l3w, cfg.kc_cat).astype(BF16)
    l1bc = np.ascontiguousarray(
        np.asarray(l1b, np.float32).reshape(cfg.kc_cat, P).T)      # [128, kc]
    l2bc = np.ascontiguousarray(
        np.asarray(l2b, np.float32).reshape(cfg.kc_cat, P).T)
    l3bb = np.tile(np.asarray(l3b, np.float32)[None, :], (P, 1))   # [128, C]

    # repeated-iota constant for 2x-mode one-hot build:
    # iotaF[p, x*KBMAX + j] = x
    KBMAX = max(cfg.wb1, cfg.wb2) * max(K_LO, K_HI)
    iotaF = np.repeat(np.arange(WIN, dtype=np.float32), KBMAX)[None, :]
    iotaF = np.ascontiguousarray(np.tile(iotaF, (P, 1))).astype(BF16)

    in_maps = []
    for i in range(cfg.n_cores):
        idx16 = {0: np.zeros((G, NW * K_LO * P), np.int16),
                 1: np.zeros((G, NW * K_HI * P), np.int16)}
        mdst = {0: np.zeros((G, P, NW * K_LO), np.float32),
                1: np.zeros((G, P, NW * K_HI), np.float32)}
        mw = {0: np.zeros((G, P, NW * K_LO), np.float32),
              1: np.zeros((G, P, NW * K_HI), np.float32)}
        for g in range(G):
            m = buckets[(i, g)]
            key = win_of[g][m] * 2 + half_flag[g][m]
            order = np.argsort(key, kind="stable")
            skey = key[order]
            cnt = np.bincount(skey, minlength=NW * 2)
            starts = np.concatenate([[0], np.cumsum(cnt)[:-1]])
            slot = np.arange(len(skey)) - starts[skey]
            il = idx_local[g][m][order]
            dw = dst_in_win[g][m][order]
            we = w_eff[g][m][order]
            swin = skey // 2
            shf = skey % 2
            for h, K in ((0, K_LO), (1, K_HI)):
                sel = shf == h
                pos = swin[sel] * (K * P) + slot[sel]     # (win, c, p) flat
                idx16[h][g][pos] = il[sel].astype(np.int16)
                c = slot[sel] // P
                p = slot[sel] % P
                mdst[h][g][p, swin[sel] * K + c] = dw[sel]
                mw[h][g][p, swin[sel] * K + c] = we[sel]

        def wrap(arr, K):
            # full-span 16-wrap idx stripes: [G, 128, NW*K*8]; any contiguous
            # 128-idx (=8 column) range can be sliced for one gather call
            out = np.zeros((G, P, NW * K * 8), np.int16)
            for g in range(G):
                wr = arr[g].reshape(-1, 16).T          # [16, NW*K*8]
                out[g] = np.tile(wr, (8, 1))
            return out

        im = {
            "xrows": xrows, "iotaF": iotaF,
            "w1c": W1c, "w2c": W2c, "l1wc": l1wc, "l2wc": l2wc,
            "l3wc": l3wc, "l1bc": l1bc, "l2bc": l2bc, "l3bb": l3bb,
            "idx_lo": wrap(idx16[0], K_LO), "idx_hi": wrap(idx16[1], K_HI),
            "mdst_lo": mdst[0].astype(BF16), "mdst_hi": mdst[1].astype(BF16),
            "mw_lo": mw[0].astype(BF16), "mw_hi": mw[1].astype(BF16),
        }
        in_maps.append(im)
    return in_maps, K_LO, K_HI


def _build(cfg: Cfg, K_LO, K_HI):
    G, NW, WIN = cfg.g_num, cfg.nwin, cfg.win
    KC = cfg.kc_cat
    HF = cfg.h_feats
    CLS = cfg.n_classes
    IF = cfg.in_feats
    KBMAX = max(cfg.wb1, cfg.wb2) * max(K_LO, K_HI)
    f32, bf16, i16 = mybir.dt.float32, mybir.dt.bfloat16, mybir.dt.int16

    nc = bacc.Bacc(num_swdge_queues=4)
    t_xr = nc.declare_dram_parameter("xrows", [cfg.rows, IF], bf16, isOutput=False)
    t_iota = nc.declare_dram_parameter("iotaF", [P, WIN * KBMAX], bf16, isOutput=False)
    t_w1 = nc.declare_dram_parameter("w1c", [P, cfg.kc_in * HF], bf16, isOutput=False)
    t_w2 = nc.declare_dram_parameter("w2c", [P, KC * HF], bf16, isOutput=False)
    t_l1w = nc.declare_dram_parameter("l1wc", [P, KC * cfg.cat], bf16, isOutput=False)
    t_l2w = nc.declare_dram_parameter("l2wc", [P, KC * cfg.cat], bf16, isOutput=False)
    t_l3w = nc.declare_dram_parameter("l3wc", [P, KC * CLS], bf16, isOutput=False)
    t_l1b = nc.declare_dram_parameter("l1bc", [P, KC], f32, isOutput=False)
    t_l2b = nc.declare_dram_parameter("l2bc", [P, KC], f32, isOutput=False)
    t_l3b = nc.declare_dram_parameter("l3bb", [P, CLS], f32, isOutput=False)
    t_ilo = nc.declare_dram_parameter("idx_lo", [G, P, NW * K_LO * 8], i16, isOutput=False)
    t_ihi = nc.declare_dram_parameter("idx_hi", [G, P, NW * K_HI * 8], i16, isOutput=False)
    t_mdl = nc.declare_dram_parameter("mdst_lo", [G, P, NW * K_LO], bf16, isOutput=False)
    t_mdh = nc.declare_dram_parameter("mdst_hi", [G, P, NW * K_HI], bf16, isOutput=False)
    t_mwl = nc.declare_dram_parameter("mw_lo", [G, P, NW * K_LO], bf16, isOutput=False)
    t_mwh = nc.declare_dram_parameter("mw_hi", [G, P, NW * K_HI], bf16, isOutput=False)
    t_out = nc.declare_dram_parameter("out", [WIN, NW * CLS], f32, isOutput=True)

    d_t2s = nc.dram_tensor("t2s", [cfg.shard_p, HF], bf16)
    d_t2f = nc.dram_tensor("t2f", [cfg.rows, HF], bf16, addr_space="Shared")

    AF = mybir.ActivationFunctionType
    qctr = [0]

    with tile.TileContext(nc) as tc:
        with (
            tc.tile_pool(name="const", bufs=1) as cp,
            tc.tile_pool(name="gath", bufs=2) as gp,
            tc.tile_pool(name="hcat", bufs=2) as hp,
            tc.tile_pool(name="dense", bufs=3) as dp,
            tc.tile_pool(name="psa", bufs=2, space="PSUM") as pm,
            tc.tile_pool(name="psb", bufs=2, space="PSUM") as pb,
        ):
            # constants
            ident = cp.tile([P, P], f32)
            make_identity(nc, ident[:])

            def const_load(t, shape, dtype):
                s = cp.tile(shape, dtype, tag=t.name + "_c")
                nc.sync.dma_start(out=s[:], in_=t[:])
                return s

            iota_sb = const_load(t_iota, [P, WIN * KBMAX], bf16)
            w1_sb = const_load(t_w1, [P, cfg.kc_in * HF], bf16)
            w2_sb = const_load(t_w2, [P, KC * HF], bf16)
            l1w_sb = const_load(t_l1w, [P, KC * cfg.cat], bf16)
            l2w_sb = const_load(t_l2w, [P, KC * cfg.cat], bf16)
            l3w_sb = const_load(t_l3w, [P, KC * CLS], bf16)
            l1b_sb = const_load(t_l1b, [P, KC], f32)
            l2b_sb = const_load(t_l2b, [P, KC], f32)
            l3b_sb = const_load(t_l3b, [P, CLS], f32)
            out_sb = cp.tile([WIN, NW * CLS], f32)

            # ------------- SpMM + dense layers, per window batch -------------
            def spmm_layer(layer2):
                # layer 1 gathers 256-wide x rows; layer 2 gathers 128-wide T2
                WB = cfg.wb2 if layer2 else cfg.wb1
                FW = HF if layer2 else IF          # gathered row width
                nb = (NW + WB - 1) // WB
                for b in range(nb):
                    w0 = b * WB
                    w1 = min(NW, w0 + WB)
                    nw = w1 - w0
                    hcat_t = {}
                    for g in range(G):
                        feats = {}
                        sels = {}
                        kbs = {}
                        for h, K, t_i, t_md, t_mw in (
                            (0, K_LO, t_ilo, t_mdl, t_mwl),
                            (1, K_HI, t_ihi, t_mdh, t_mwh),
                        ):
                            KB = nw * K
                            kbs[h] = KB
                            idx_t = gp.tile([P, WB * K * 8], i16, tag=f"idx{h}")
                            nc.sync.dma_start(
                                out=idx_t[:, :KB * 8],
                                in_=t_i[g][:, w0 * K * 8:w1 * K * 8])
                            md_t = gp.tile([P, WB * K], bf16, tag=f"md{h}")
                            nc.sync.dma_start(out=md_t[:, :KB],
                                              in_=t_md[g][:, w0 * K:w1 * K])
                            mw_t = gp.tile([P, WB * K], bf16, tag=f"mw{h}")
                            nc.sync.dma_start(out=mw_t[:, :KB],
                                              in_=t_mw[g][:, w0 * K:w1 * K])
                            ft = gp.tile([P, WB * K * FW], bf16,
                                         tag=f"ft{h}")
                            table = d_t2f if layer2 else t_xr
                            for j in range(0, KB, GCH):
                                gl = min(GCH, KB - j)
                                nc.gpsimd.dma_gather(
                                    out_ap=ft[:, j * FW:(j + gl) * FW].rearrange(
                                        "p (k f) -> p k f", f=FW),
                                    in_ap=table[(cfg.half if h else 0):
                                                (cfg.rows if h else cfg.half), :],
                                    idxs_ap=idx_t[:, j * 8:(j + gl) * 8],
                                    num_idxs=gl * P, num_idxs_reg=gl * P,
                                    elem_size=FW, elem_step=FW,
                                    queue_num=qctr[0] % 4,
                                )
                                qctr[0] += 1
                            # one-hot selection in 2x DVE mode:
                            # st[p, x, j] = (md[p,j] == x) * mw[p,j]
                            st = gp.tile([P, WIN * WB * K], bf16, tag=f"st{h}")
                            st3 = st[:, :WIN * KB].rearrange(
                                "p (x k) -> p x k", k=KB)
                            mda = md_t[:]
                            md_b = bass.AP(mda.tensor, mda.offset,
                                           [list(mda.ap[0]), [0, WIN], [1, KB]])
                            ioa = iota_sb[:]
                            io_b = bass.AP(ioa.tensor, ioa.offset,
                                           [list(ioa.ap[0]), [KBMAX, WIN],
                                            [1, KB]])
                            nc.vector.tensor_tensor(
                                out=st3, in0=md_b, in1=io_b,
                                op=mybir.AluOpType.is_equal)
                            mwa = mw_t[:]
                            mw_b = bass.AP(mwa.tensor, mwa.offset,
                                           [list(mwa.ap[0]), [0, WIN], [1, KB]])
                            nc.vector.tensor_tensor(
                                out=st3, in0=st3, in1=mw_b,
                                op=mybir.AluOpType.mult)
                            feats[h] = ft
                            sels[h] = st
                        for wi in range(w0, w1):
                            dw = wi - w0
                            tot = K_LO + K_HI
                            if layer2:
                                pst = pm.tile([P, WIN], f32, tag="agga")
                                ps = pst[:]
                                ci = 0
                                for h, K in ((0, K_LO), (1, K_HI)):
                                    ft, st = feats[h], sels[h]
                                    KB = kbs[h]
                                    sta = st[:]
                                    for c in range(K):
                                        cc = dw * K + c
                                        rhs = bass.AP(
                                            sta.tensor, sta.offset + cc,
                                            [list(sta.ap[0]), [KB, WIN]])
                                        nc.tensor.matmul(
                                            out=ps,
                                            lhsT=ft[:, cc * HF:(cc + 1) * HF],
                                            rhs=rhs,
                                            start=(ci == 0), stop=(ci == tot - 1))
                                        ci += 1
                                hc = hp.tile([P, WIN], bf16, tag=f"hc{dw}_{g}")
                                nc.scalar.activation(hc[:], ps, AF.Relu)
                            else:
                                # aggregate in 256-d input space (two psum
                                # banks), then project with W1
                                psa_t = pm.tile([P, WIN], f32, tag="agga")
                                psb_t = pm.tile([P, WIN], f32, tag="aggb")
                                psa = psa_t[:]
                                psb = psb_t[:]
                                ci = 0
                                for h, K in ((0, K_LO), (1, K_HI)):
                                    ft, st = feats[h], sels[h]
                                    KB = kbs[h]
                                    sta = st[:]
                                    for c in range(K):
                                        cc = dw * K + c
                                        rhs = bass.AP(
                                            sta.tensor, sta.offset + cc,
                                            [list(sta.ap[0]), [KB, WIN]])
                                        st_f = (ci == 0)
                                        sp_f = (ci == tot - 1)
                                        nc.tensor.matmul(
                                            out=psa,
                                            lhsT=ft[:, cc * IF:cc * IF + P],
                                            rhs=rhs, start=st_f, stop=sp_f)
                                        nc.tensor.matmul(
                                            out=psb,
                                            lhsT=ft[:, cc * IF + P:(cc + 1) * IF],
                                            rhs=rhs, start=st_f, stop=sp_f)
                                        ci += 1
                                sa = dp.tile([P, WIN], bf16, tag="sagga")
                                nc.scalar.activation(sa[:], psa, AF.Copy)
                                sb = dp.tile([P, WIN], bf16, tag="saggb")
                                nc.scalar.activation(sb[:], psb, AF.Copy)
                                q = pm.tile([P, WIN], f32, tag="mlp")
                                nc.tensor.matmul(
                                    out=q[:], lhsT=w1_sb[:, :HF], rhs=sa[:],
                                    start=True, stop=False)
                                nc.tensor.matmul(
                                    out=q[:], lhsT=w1_sb[:, HF:2 * HF],
                                    rhs=sb[:], start=False, stop=True)
                                hc = hp.tile([P, WIN], bf16, tag=f"hc{dw}_{g}")
                                nc.scalar.activation(hc[:], q[:], AF.Relu)
                            hcat_t[(wi, g)] = hc
                    for wi in range(w0, w1):
                        hcat = [hcat_t[(wi, g)] for g in range(G)]
                        if not layer2:
                            def mlp(ws, bs, ins, name):
                                outs = []
                                for fc in range(KC):
                                    ps = pm.tile([P, WIN], f32, tag="mlp")
                                    for kc in range(KC):
                                        nc.tensor.matmul(
                                            out=ps[:],
                                            lhsT=ws[:, (kc * KC + fc) * P:
                                                    (kc * KC + fc + 1) * P],
                                            rhs=ins[kc][:],
                                            start=(kc == 0), stop=(kc == KC - 1))
                                    o = dp.tile([P, WIN], bf16,
                                                tag=f"mlpo{name}{fc}")
                                    nc.scalar.activation(o[:], ps[:], AF.Relu,
                                                         bias=bs[:, fc:fc + 1])
                                    outs.append(o)
                                return outs
                            hl1 = mlp(l1w_sb, l1b_sb, hcat, "a")
                            hl2 = mlp(l2w_sb, l2b_sb, hl1, "b")
                            p2 = pb.tile([P, WIN], f32, tag="misc")
                            for kc in range(KC):
                                nc.tensor.matmul(
                                    out=p2[:],
                                    lhsT=w2_sb[:, kc * HF:(kc + 1) * HF],
                                    rhs=hl2[kc][:],
                                    start=(kc == 0), stop=(kc == KC - 1))
                            p2s = dp.tile([P, WIN], f32, tag="p2s")
                            nc.scalar.activation(p2s[:], p2[:], AF.Copy)
                            p2t = pb.tile([WIN, P], f32, tag="misc")
                            nc.tensor.transpose(p2t[:], p2s[:], ident[:])
                            h2r = dp.tile([WIN, HF], bf16, tag="h2r")
                            nc.scalar.activation(h2r[:], p2t[:], AF.Copy)
                            nc.sync.dma_start(
                                out=d_t2s[wi * WIN:(wi + 1) * WIN, :], in_=h2r[:])
                        else:
                            ps = pb.tile([WIN, CLS], f32, tag="misc")
                            for kc in range(KC):
                                nc.tensor.matmul(
                                    out=ps[:],
                                    lhsT=hcat[kc][:],
                                    rhs=l3w_sb[:, kc * CLS:(kc + 1) * CLS],
                                    start=(kc == 0), stop=(kc == KC - 1))
                            nc.vector.tensor_tensor(
                                out=out_sb[:, wi * CLS:(wi + 1) * CLS],
                                in0=ps[:], in1=l3b_sb[:WIN, :],
                                op=mybir.AluOpType.add)

            spmm_layer(layer2=False)

            tc.strict_bb_all_engine_barrier()
            nc.gpsimd.collective_compute(
                "AllGather", mybir.AluOpType.bypass,
                ins=[d_t2s[:]], outs=[d_t2f[:]],
                replica_groups=[list(range(cfg.n_cores))],
            )
            tc.strict_bb_all_engine_barrier()

            spmm_layer(layer2=True)

            nc.sync.dma_start(out=t_out[:], in_=out_sb[:])
    nc.finalize()
    return nc


def _run(cfg: Cfg, inputs: dict, trace: bool = False):
    _install_ntff_hook()
    from concourse import bass_utils
    bass_utils.upload_artifacts = lambda d: "local://skipped"
    from concourse.bass_utils import run_bass_kernel_spmd

    in_maps, K_LO, K_HI = _prep_inputs(cfg, **inputs)
    nc = _build(cfg, K_LO, K_HI)
    res = run_bass_kernel_spmd(nc, in_maps, list(range(cfg.n_cores)),
                               trace=trace)
    outs = []
    for i in range(cfg.n_cores):
        o = res.results[i]["out"]                     # [WIN, nwin*CLS]
        o = o.reshape(cfg.win, cfg.nwin, cfg.n_classes).transpose(1, 0, 2)
        outs.append(o.reshape(cfg.shard_p, cfg.n_classes)[:cfg.shard])
    full = np.concatenate(outs, axis=0)
    return full, res.exec_time_ns


def kernel(**inputs) -> np.ndarray:
    cfg = Cfg()
    out, _ = _run(cfg, inputs, trace=False)
    return out.astype(np.float32)


# revision 23
# speedup vs baseline: 1.3034x; 1.3034x over previous
"""Trainium2 Bass kernel for the GWNN2 GNN (4-graph GraphConv x2 + MLP).

v2 strategy (8 NeuronCores, dst-sharded):
  * all deg_in/deg_out normalization folded into per-edge one-hot weights
    (host side) -> ONE shared projection table per layer:
       T1 = x @ W1        [50176, 128] bf16   (replicated compute, no collective)
       T2 = h @ W2        [50176, 128] bf16   (sharded compute + small AllGather)
  * SpMM per dst window of 128 nodes: one merged dma_gather per
    (graph, table-half, batch of 7 windows) = 112 calls total,
    one-hot selection built on DVE in 2x mode (st layout [P, WIN, KB]
    keeps every operand's last dim stride-1), PSUM-accumulated matmul
  * dense l1/l2 + W2 fused per window between the two conv layers
"""
import sys
import types
from dataclasses import dataclass

if "/opt/trn_rl_repo" not in sys.path:
    sys.path.insert(0, "/opt/trn_rl_repo")

import numpy as np
import ml_dtypes

import concourse.bass as bass
import concourse.bacc as bacc
import concourse.mybir as mybir
import concourse.tile as tile
from concourse.masks import make_identity

BF16 = ml_dtypes.bfloat16
P = 128
ONEHOT_2X = True


def _install_ntff_hook():
    """Make trace=True usable under axon (antenv.axon_hooks may be absent)."""
    try:
        import antenv
        if "antenv.axon_hooks" in sys.modules:
            return
        m = types.ModuleType("antenv.axon_hooks")
        box = [None]
        m.set_axon_ntff_profile_hook = lambda h: box.__setitem__(0, h)
        m.get_axon_ntff_profile_hook = lambda: box[0]
        sys.modules["antenv.axon_hooks"] = m
        antenv.axon_hooks = m
        try:
            from trn_agent_boot.trn_boot import _ntff_profile_via_ctypes
            hook = _ntff_profile_via_ctypes("/opt/axon/libaxon_pjrt.so")
            if hook is not None:
                m.set_axon_ntff_profile_hook(hook)
        except Exception:
            pass
    except Exception:
        pass


@dataclass
class Cfg:
    n_nodes: int = 50000
    g_num: int = 4
    in_feats: int = 256
    h_feats: int = 128          # table row width (must be 128)
    n_classes: int = 40
    n_cores: int = 8
    win: int = 128              # dst nodes per SpMM window
    win_batch: int = 7          # windows per merged dma_gather call

    @property
    def shard(self):
        return self.n_nodes // self.n_cores

    @property
    def shard_p(self):          # padded shard rows
        return ((self.shard + P - 1) // P) * P

    @property
    def rows(self):             # padded table rows
        return self.shard_p * self.n_cores

    @property
    def half(self):
        return self.rows // 2

    @property
    def nwin(self):
        return self.shard_p // self.win

    @property
    def nbatch(self):
        return (self.nwin + self.win_batch - 1) // self.win_batch

    @property
    def cat(self):
        return self.h_feats * self.g_num

    @property
    def kc_cat(self):           # 128-chunks in cat dim
        return self.cat // P

    @property
    def kc_in(self):
        return self.in_feats // P

    @property
    def ntile(self):            # node tiles (128) in full padded table
        return self.rows // P


def _prep_inputs(cfg: Cfg, in_feat, src, dst, w, W1, W2, l1w, l1b, l2w, l2b,
                 l3w, l3b):
    """Host-side sharding/packing. Returns (in_maps, K_LO, K_HI)."""
    N, G = cfg.n_nodes, cfg.g_num
    SH, SHP = cfg.shard, cfg.shard_p
    NW, WIN, WB = cfg.nwin, cfg.win, cfg.win_batch
    HALF = cfg.half
    src = np.asarray(src).astype(np.int64)
    dst = np.asarray(dst).astype(np.int64)
    w = np.asarray(w, dtype=np.float32)
    in_feat = np.asarray(in_feat, dtype=np.float32)

    deg_out = np.empty((G, N), np.float32)
    deg_in = np.empty((G, N), np.float32)
    for g in range(G):
        deg_out[g] = np.clip(np.bincount(src[g], minlength=N), 1.0, None) ** -0.5
        deg_in[g] = np.clip(np.bincount(dst[g], minlength=N), 1.0, None) ** -0.5

    src_pad = (src // SH) * SHP + (src % SH)          # padded table row
    half_flag = (src_pad >= HALF).astype(np.int64)
    idx_local = (src_pad - half_flag * HALF).astype(np.int64)

    core_of = dst // SH
    dst_loc = dst % SH
    win_of = dst_loc // WIN
    dst_in_win = (dst_loc % WIN).astype(np.float32)

    # first pass: counts to fix K_LO / K_HI globally (one SPMD NEFF)
    maxlo = maxhi = 1
    buckets = {}
    for i in range(cfg.n_cores):
        for g in range(G):
            m = core_of[g] == i
            key = win_of[g][m] * 2 + half_flag[g][m]
            cnt = np.bincount(key, minlength=NW * 2)
            maxlo = max(maxlo, int(cnt[0::2].max()))
            maxhi = max(maxhi, int(cnt[1::2].max()))
            buckets[(i, g)] = m
    K_LO = (maxlo + P - 1) // P
    K_HI = (maxhi + P - 1) // P

    # all normalization folded into the per-edge selection weight
    w_eff = np.empty((G, src.shape[1]), np.float32)
    for g in range(G):
        w_eff[g] = w[g] * deg_in[g][dst[g]] * deg_out[g][src[g]]

    # replicated phase-1 inputs (identical for every core): full x, packed
    # transposed per tile for lhsT use
    xpad = np.zeros((cfg.rows, cfg.in_feats), np.float32)
    for i in range(cfg.n_cores):
        xpad[i * SHP:i * SHP + SH] = in_feat[i * SH:(i + 1) * SH]
    xt4 = xpad.reshape(cfg.ntile, P, cfg.kc_in, P)     # (t, n, kc, k)
    xtiles = np.ascontiguousarray(xt4.transpose(3, 0, 2, 1)).reshape(
        P, cfg.ntile, cfg.kc_in * P).astype(BF16)

    def pack_lhsT(W, kc):
        Wr = np.asarray(W, np.float32).reshape(kc, P, -1)   # (kc, k, fout)
        return np.ascontiguousarray(Wr.transpose(1, 0, 2)).reshape(P, -1)

    W1c = pack_lhsT(W1, cfg.kc_in).astype(BF16)
    W2c = pack_lhsT(W2, cfg.kc_cat).astype(BF16)
    l1wc = pack_lhsT(l1w, cfg.kc_cat).astype(BF16)
    l2wc = pack_lhsT(l2w, cfg.kc_cat).astype(BF16)
    l3wc = pack_lhsT(l3w, cfg.kc_cat).astype(BF16)
    l1bc = np.ascontiguousarray(
        np.asarray(l1b, np.float32).reshape(cfg.kc_cat, P).T)      # [128, kc]
    l2bc = np.ascontiguousarray(
        np.asarray(l2b, np.float32).reshape(cfg.kc_cat, P).T)
    l3bb = np.tile(np.asarray(l3b, np.float32)[None, :], (P, 1))   # [128, C]

    # repeated-iota constant for 2x-mode one-hot build:
    # iotaF[p, x*KBMAX + j] = x
    KBMAX = WB * max(K_LO, K_HI)
    if ONEHOT_2X:
        iotaF = np.repeat(np.arange(WIN, dtype=np.float32), KBMAX)[None, :]
        iotaF = np.ascontiguousarray(np.tile(iotaF, (P, 1))).astype(BF16)
    else:
        iotaF = np.zeros((P, WIN * KBMAX), np.float32)
        iotaF[:, :WIN] = np.arange(WIN, dtype=np.float32)[None, :]
        iotaF = iotaF.astype(BF16)

    NB = cfg.nbatch
    in_maps = []
    for i in range(cfg.n_cores):
        idx16 = {0: np.zeros((G, NW * K_LO * P), np.int16),
                 1: np.zeros((G, NW * K_HI * P), np.int16)}
        mdst = {0: np.zeros((G, P, NW * K_LO), np.float32),
                1: np.zeros((G, P, NW * K_HI), np.float32)}
        mw = {0: np.zeros((G, P, NW * K_LO), np.float32),
              1: np.zeros((G, P, NW * K_HI), np.float32)}
        for g in range(G):
            m = buckets[(i, g)]
            key = win_of[g][m] * 2 + half_flag[g][m]
            order = np.argsort(key, kind="stable")
            skey = key[order]
            cnt = np.bincount(skey, minlength=NW * 2)
            starts = np.concatenate([[0], np.cumsum(cnt)[:-1]])
            slot = np.arange(len(skey)) - starts[skey]
            il = idx_local[g][m][order]
            dw = dst_in_win[g][m][order]
            we = w_eff[g][m][order]
            swin = skey // 2
            shf = skey % 2
            for h, K in ((0, K_LO), (1, K_HI)):
                sel = shf == h
                pos = swin[sel] * (K * P) + slot[sel]     # (win, c, p) flat
                idx16[h][g][pos] = il[sel].astype(np.int16)
                c = slot[sel] // P
                p = slot[sel] % P
                mdst[h][g][p, swin[sel] * K + c] = dw[sel]
                mw[h][g][p, swin[sel] * K + c] = we[sel]

        def wrap(arr, K):
            # one contiguous idx stripe per (g, batch): [G, NB, 128, WB*K*8]
            out = np.zeros((G, NB, P, WB * K * 8), np.int16)
            for g in range(G):
                for b in range(NB):
                    w0 = b * WB
                    w1 = min(NW, w0 + WB)
                    fl = arr[g][w0 * K * P: w1 * K * P]
                    wr = fl.reshape(-1, 16).T          # [16, n]
                    out[g, b][:, :(w1 - w0) * K * 8] = np.tile(wr, (8, 1))
            return out

        im = {
            "xtiles": xtiles, "iotaF": iotaF,
            "w1c": W1c, "w2c": W2c, "l1wc": l1wc, "l2wc": l2wc,
            "l3wc": l3wc, "l1bc": l1bc, "l2bc": l2bc, "l3bb": l3bb,
            "idx_lo": wrap(idx16[0], K_LO), "idx_hi": wrap(idx16[1], K_HI),
            "mdst_lo": mdst[0].astype(BF16), "mdst_hi": mdst[1].astype(BF16),
            "mw_lo": mw[0].astype(BF16), "mw_hi": mw[1].astype(BF16),
        }
        in_maps.append(im)
    return in_maps, K_LO, K_HI


def _build(cfg: Cfg, K_LO, K_HI):
    G, NW, WIN, WB = cfg.g_num, cfg.nwin, cfg.win, cfg.win_batch
    NB = cfg.nbatch
    KC = cfg.kc_cat
    HF = cfg.h_feats
    CLS = cfg.n_classes
    KBMAX = WB * max(K_LO, K_HI)
    f32, bf16, i16, i32 = (mybir.dt.float32, mybir.dt.bfloat16,
                           mybir.dt.int16, mybir.dt.int32)

    nc = bacc.Bacc(num_swdge_queues=4)
    t_xt = nc.declare_dram_parameter("xtiles", [P, cfg.ntile, cfg.kc_in * P], bf16, isOutput=False)
    t_iota = nc.declare_dram_parameter("iotaF", [P, WIN * KBMAX], bf16, isOutput=False)
    t_w1 = nc.declare_dram_parameter("w1c", [P, cfg.kc_in * HF], bf16, isOutput=False)
    t_w2 = nc.declare_dram_parameter("w2c", [P, KC * HF], bf16, isOutput=False)
    t_l1w = nc.declare_dram_parameter("l1wc", [P, KC * cfg.cat], bf16, isOutput=False)
    t_l2w = nc.declare_dram_parameter("l2wc", [P, KC * cfg.cat], bf16, isOutput=False)
    t_l3w = nc.declare_dram_parameter("l3wc", [P, KC * CLS], bf16, isOutput=False)
    t_l1b = nc.declare_dram_parameter("l1bc", [P, KC], f32, isOutput=False)
    t_l2b = nc.declare_dram_parameter("l2bc", [P, KC], f32, isOutput=False)
    t_l3b = nc.declare_dram_parameter("l3bb", [P, CLS], f32, isOutput=False)
    t_ilo = nc.declare_dram_parameter("idx_lo", [G, NB, P, WB * K_LO * 8], i16, isOutput=False)
    t_ihi = nc.declare_dram_parameter("idx_hi", [G, NB, P, WB * K_HI * 8], i16, isOutput=False)
    t_mdl = nc.declare_dram_parameter("mdst_lo", [G, P, NW * K_LO], bf16, isOutput=False)
    t_mdh = nc.declare_dram_parameter("mdst_hi", [G, P, NW * K_HI], bf16, isOutput=False)
    t_mwl = nc.declare_dram_parameter("mw_lo", [G, P, NW * K_LO], bf16, isOutput=False)
    t_mwh = nc.declare_dram_parameter("mw_hi", [G, P, NW * K_HI], bf16, isOutput=False)
    t_out = nc.declare_dram_parameter("out", [WIN, NW * CLS], f32, isOutput=True)

    d_t1 = nc.dram_tensor("t1", [cfg.rows, HF], bf16)
    d_t2s = nc.dram_tensor("t2s", [cfg.shard_p, HF], bf16)
    d_t2f = nc.dram_tensor("t2f", [cfg.rows, HF], bf16, addr_space="Shared")

    AF = mybir.ActivationFunctionType
    qctr = [0]

    with tile.TileContext(nc) as tc:
        with (
            tc.tile_pool(name="const", bufs=1) as cp,
            tc.tile_pool(name="x", bufs=3) as xp,
            tc.tile_pool(name="gath", bufs=2) as gp,
            tc.tile_pool(name="hcat", bufs=2) as hp,
            tc.tile_pool(name="dense", bufs=3) as dp,
            tc.tile_pool(name="psa", bufs=2, space="PSUM") as pm,
            tc.tile_pool(name="psb", bufs=2, space="PSUM") as pb,
        ):
            # constants
            ident = cp.tile([P, P], f32)
            make_identity(nc, ident[:])

            def const_load(t, shape, dtype):
                s = cp.tile(shape, dtype, tag=t.name + "_c")
                nc.sync.dma_start(out=s[:], in_=t[:])
                return s

            iota_sb = const_load(t_iota, [P, WIN * KBMAX], bf16)
            w1_sb = const_load(t_w1, [P, cfg.kc_in * HF], bf16)
            w2_sb = const_load(t_w2, [P, KC * HF], bf16)
            l1w_sb = const_load(t_l1w, [P, KC * cfg.cat], bf16)
            l2w_sb = const_load(t_l2w, [P, KC * cfg.cat], bf16)
            l3w_sb = const_load(t_l3w, [P, KC * CLS], bf16)
            l1b_sb = const_load(t_l1b, [P, KC], f32)
            l2b_sb = const_load(t_l2b, [P, KC], f32)
            l3b_sb = const_load(t_l3b, [P, CLS], f32)
            out_sb = cp.tile([WIN, NW * CLS], f32)

            # ---------------- phase 1: replicated T1 = x @ W1 ----------------
            # batched tile-group DMAs keep phase 1 off the sync-queue
            # dispatch-rate limit; psum rotates across the two idle pools
            TG = 8                       # tiles per DMA group
            KIP = cfg.kc_in * P
            for t0 in range(0, cfg.ntile, TG):
                tg = min(TG, cfg.ntile - t0)
                xt = xp.tile([P, TG * KIP], bf16, tag="xt")
                nc.sync.dma_start(
                    out=xt[:, :tg * KIP].rearrange("p (t f) -> p t f", f=KIP),
                    in_=t_xt[:, t0:t0 + tg, :])
                h1row = xp.tile([P, TG * HF], bf16, tag="h1row")
                for ti in range(tg):
                    pool = pb if ti % 2 == 0 else pm
                    ptag = "misc" if ti % 2 == 0 else "mlp"
                    q1 = pool.tile([P, HF], f32, tag=ptag)
                    xo = ti * KIP
                    for kc in range(cfg.kc_in):
                        nc.tensor.matmul(
                            out=q1[:], lhsT=xt[:, xo + kc * P:xo + (kc + 1) * P],
                            rhs=w1_sb[:, kc * HF:(kc + 1) * HF],
                            start=(kc == 0), stop=(kc == cfg.kc_in - 1))
                    if ti % 2 == 0:
                        nc.scalar.activation(
                            h1row[:, ti * HF:(ti + 1) * HF], q1[:], AF.Copy)
                    else:
                        nc.vector.tensor_copy(
                            h1row[:, ti * HF:(ti + 1) * HF], q1[:])
                # store rows (t, p, f) from sbuf layout (p, t, f)
                h1a = h1row[:]
                t1a = d_t1[:]
                out_ap = bass.AP(t1a.tensor, t1a.offset + t0 * P * HF,
                                 [[HF, P], [P * HF, tg], [1, HF]])
                in_ap = bass.AP(h1a.tensor, h1a.offset,
                                [list(h1a.ap[0]), [HF, tg], [1, HF]])
                nc.sync.dma_start(out=out_ap, in_=in_ap)

            tc.strict_bb_all_engine_barrier()

            # ------------- SpMM + dense layers, per window batch -------------
            def spmm_layer(table, layer2):
                for b in range(NB):
                    w0 = b * WB
                    w1 = min(NW, w0 + WB)
                    nw = w1 - w0
                    hcat_t = {}
                    for g in range(G):
                        feats = {}
                        sels = {}
                        kbs = {}
                        for h, K, t_i, t_md, t_mw in (
                            (0, K_LO, t_ilo, t_mdl, t_mwl),
                            (1, K_HI, t_ihi, t_mdh, t_mwh),
                        ):
                            KB = nw * K
                            kbs[h] = KB
                            idx_t = gp.tile([P, WB * K * 8], i16, tag=f"idx{h}")
                            nc.sync.dma_start(
                                out=idx_t[:, :KB * 8],
                                in_=t_i[g, b][:, :KB * 8])
                            md_t = gp.tile([P, WB * K], bf16, tag=f"md{h}")
                            nc.sync.dma_start(out=md_t[:, :KB],
                                              in_=t_md[g][:, w0 * K:w1 * K])
                            mw_t = gp.tile([P, WB * K], bf16, tag=f"mw{h}")
                            nc.sync.dma_start(out=mw_t[:, :KB],
                                              in_=t_mw[g][:, w0 * K:w1 * K])
                            ft = gp.tile([P, WB * K * HF], bf16, tag=f"ft{h}")
                            GCH = 8        # chunks (of 128 idx) per gather call (HW ring limit)
                            for j in range(0, KB, GCH):
                                gl = min(GCH, KB - j)
                                nc.gpsimd.dma_gather(
                                    out_ap=ft[:, j * HF:(j + gl) * HF].rearrange(
                                        "p (k f) -> p k f", f=HF),
                                    in_ap=table[(cfg.half if h else 0):
                                                (cfg.rows if h else cfg.half), :],
                                    idxs_ap=idx_t[:, j * 8:(j + gl) * 8],
                                    num_idxs=gl * P, num_idxs_reg=gl * P,
                                    elem_size=HF, elem_step=HF,
                                    queue_num=qctr[0] % 4,
                                )
                                qctr[0] += 1
                            st = gp.tile([P, WIN * WB * K], bf16, tag=f"st{h}")
                            if ONEHOT_2X:
                                # st[p, x, j] = (md[p,j] == x) * mw[p,j]
                                st3 = st[:, :WIN * KB].rearrange(
                                    "p (x k) -> p x k", k=KB)
                                mda = md_t[:]
                                md_b = bass.AP(mda.tensor, mda.offset,
                                               [list(mda.ap[0]), [0, WIN],
                                                [1, KB]])
                                ioa = iota_sb[:]
                                io_b = bass.AP(ioa.tensor, ioa.offset,
                                               [list(ioa.ap[0]), [KBMAX, WIN],
                                                [1, KB]])
                                nc.vector.tensor_tensor(
                                    out=st3, in0=md_b, in1=io_b,
                                    op=mybir.AluOpType.is_equal)
                                mwa = mw_t[:]
                                mw_b = bass.AP(mwa.tensor, mwa.offset,
                                               [list(mwa.ap[0]), [0, WIN],
                                                [1, KB]])
                                nc.vector.tensor_tensor(
                                    out=st3, in0=st3, in1=mw_b,
                                    op=mybir.AluOpType.mult)
                            else:
                                # baseline layout: st[p, j, x]
                                st3 = st[:, :KB * WIN].rearrange(
                                    "p (k x) -> p k x", x=WIN)
                                ioa = iota_sb[:]
                                io_b = bass.AP(ioa.tensor, ioa.offset,
                                               [list(ioa.ap[0]), [0, KB],
                                                [1, WIN]])
                                nc.vector.tensor_tensor(
                                    out=st3,
                                    in0=md_t[:, :KB].to_broadcast(
                                        [P, KB, WIN]),
                                    in1=io_b,
                                    op=mybir.AluOpType.is_equal)
                                nc.vector.tensor_tensor(
                                    out=st3, in0=st3,
                                    in1=mw_t[:, :KB].to_broadcast(
                                        [P, KB, WIN]),
                                    op=mybir.AluOpType.mult)
                            feats[h] = ft
                            sels[h] = st
                        for wi in range(w0, w1):
                            dw = wi - w0
                            ps = pm.tile([P, WIN], f32, tag="agg")
                            tot = K_LO + K_HI
                            ci = 0
                            for h, K in ((0, K_LO), (1, K_HI)):
                                ft, st = feats[h], sels[h]
                                KB = kbs[h]
                                sta = st[:]
                                for c in range(K):
                                    cc = dw * K + c
                                    if ONEHOT_2X:
                                        rhs = bass.AP(
                                            sta.tensor, sta.offset + cc,
                                            [list(sta.ap[0]), [KB, WIN]])
                                    else:
                                        rhs = st[:, cc * WIN:(cc + 1) * WIN]
                                    nc.tensor.matmul(
                                        out=ps[:],
                                        lhsT=ft[:, cc * HF:(cc + 1) * HF],
                                        rhs=rhs,
                                        start=(ci == 0), stop=(ci == tot - 1))
                                    ci += 1
                            hc = hp.tile([P, WIN], bf16, tag=f"hc{dw}_{g}")
                            nc.scalar.activation(hc[:], ps[:], AF.Relu)
                            hcat_t[(wi, g)] = hc
                    for wi in range(w0, w1):
                        hcat = [hcat_t[(wi, g)] for g in range(G)]
                        if not layer2:
                            def mlp(ws, bs, ins, name):
                                outs = []
                                for fc in range(KC):
                                    ps = pm.tile([P, WIN], f32, tag="mlp")
                                    for kc in range(KC):
                                        nc.tensor.matmul(
                                            out=ps[:],
                                            lhsT=ws[:, (kc * KC + fc) * P:
                                                    (kc * KC + fc + 1) * P],
                                            rhs=ins[kc][:],
                                            start=(kc == 0), stop=(kc == KC - 1))
                                    o = dp.tile([P, WIN], bf16,
                                                tag=f"mlpo{name}{fc}")
                                    nc.scalar.activation(o[:], ps[:], AF.Relu,
                                                         bias=bs[:, fc:fc + 1])
                                    outs.append(o)
                                return outs
                            hl1 = mlp(l1w_sb, l1b_sb, hcat, "a")
                            hl2 = mlp(l2w_sb, l2b_sb, hl1, "b")
                            p2 = pb.tile([P, WIN], f32, tag="misc")
                            for kc in range(KC):
                                nc.tensor.matmul(
                                    out=p2[:],
                                    lhsT=w2_sb[:, kc * HF:(kc + 1) * HF],
                                    rhs=hl2[kc][:],
                                    start=(kc == 0), stop=(kc == KC - 1))
                            p2s = dp.tile([P, WIN], f32, tag="p2s")
                            nc.scalar.activation(p2s[:], p2[:], AF.Copy)
                            p2t = pb.tile([WIN, P], f32, tag="misc")
                            nc.tensor.transpose(p2t[:], p2s[:], ident[:])
                            h2r = dp.tile([WIN, HF], bf16, tag="h2r")
                            nc.scalar.activation(h2r[:], p2t[:], AF.Copy)
                            nc.sync.dma_start(
                                out=d_t2s[wi * WIN:(wi + 1) * WIN, :], in_=h2r[:])
                        else:
                            ps = pb.tile([WIN, CLS], f32, tag="misc")
                            for kc in range(KC):
                                nc.tensor.matmul(
                                    out=ps[:],
                                    lhsT=hcat[kc][:],
                                    rhs=l3w_sb[:, kc * CLS:(kc + 1) * CLS],
                                    start=(kc == 0), stop=(kc == KC - 1))
                            nc.vector.tensor_tensor(
                                out=out_sb[:, wi * CLS:(wi + 1) * CLS],
                                in0=ps[:], in1=l3b_sb[:WIN, :],
                                op=mybir.AluOpType.add)

            spmm_layer(d_t1, layer2=False)

            tc.strict_bb_all_engine_barrier()
            nc.gpsimd.collective_compute(
                "AllGather", mybir.AluOpType.bypass,
                ins=[d_t2s[:]], outs=[d_t2f[:]],
                replica_groups=[list(range(cfg.n_cores))],
            )
            tc.strict_bb_all_engine_barrier()

            spmm_layer(d_t2f, layer2=True)

            nc.sync.dma_start(out=t_out[:], in_=out_sb[:])
    nc.finalize()
    return nc


def _run(cfg: Cfg, inputs: dict, trace: bool = False):
    _install_ntff_hook()
    from concourse import bass_utils
    bass_utils.upload_artifacts = lambda d: "local://skipped"
    from concourse.bass_utils import run_bass_kernel_spmd

    in_maps, K_LO, K_HI = _prep_inputs(cfg, **inputs)
    nc = _build(cfg, K_LO, K_HI)
    res = run_bass_kernel_spmd(nc, in_maps, list(range(cfg.n_cores)),
                               trace=trace)
    outs = []
    for i in range(cfg.n_cores):
        o = res.results[i]["out"]                     # [WIN, nwin*CLS]
        o = o.reshape(cfg.win, cfg.nwin, cfg.n_classes).transpose(1, 0, 2)
        outs.append(o.reshape(cfg.shard_p, cfg.n_classes)[:cfg.shard])
    full = np.concatenate(outs, axis=0)
    return full, res.exec_time_ns


def kernel(**inputs) -> np.ndarray:
    cfg = Cfg()
    out, _ = _run(cfg, inputs, trace=False)
    return out.astype(np.float32)


# revision 25
# speedup vs baseline: 1.3169x; 1.0103x over previous
"""Trainium2 Bass kernel for the GWNN2 GNN (4-graph GraphConv x2 + MLP).

v2 strategy (8 NeuronCores, dst-sharded):
  * all deg_in/deg_out normalization folded into per-edge one-hot weights
    (host side) -> ONE shared projection table per layer:
       T1 = x @ W1        [50176, 128] bf16   (replicated compute, no collective)
       T2 = h @ W2        [50176, 128] bf16   (sharded compute + small AllGather)
  * SpMM per dst window of 128 nodes: one merged dma_gather per
    (graph, table-half, batch of 7 windows) = 112 calls total,
    one-hot selection built on DVE in 2x mode (st layout [P, WIN, KB]
    keeps every operand's last dim stride-1), PSUM-accumulated matmul
  * dense l1/l2 + W2 fused per window between the two conv layers
"""
import sys
import types
from dataclasses import dataclass

if "/opt/trn_rl_repo" not in sys.path:
    sys.path.insert(0, "/opt/trn_rl_repo")

import numpy as np
import ml_dtypes

import concourse.bass as bass
import concourse.bacc as bacc
import concourse.mybir as mybir
import concourse.tile as tile
from concourse.masks import make_identity

BF16 = ml_dtypes.bfloat16
P = 128
ONEHOT_2X = True


def _install_ntff_hook():
    """Make trace=True usable under axon (antenv.axon_hooks may be absent)."""
    try:
        import antenv
        if "antenv.axon_hooks" in sys.modules:
            return
        m = types.ModuleType("antenv.axon_hooks")
        box = [None]
        m.set_axon_ntff_profile_hook = lambda h: box.__setitem__(0, h)
        m.get_axon_ntff_profile_hook = lambda: box[0]
        sys.modules["antenv.axon_hooks"] = m
        antenv.axon_hooks = m
        try:
            from trn_agent_boot.trn_boot import _ntff_profile_via_ctypes
            hook = _ntff_profile_via_ctypes("/opt/axon/libaxon_pjrt.so")
            if hook is not None:
                m.set_axon_ntff_profile_hook(hook)
        except Exception:
            pass
    except Exception:
        pass


@dataclass
class Cfg:
    n_nodes: int = 50000
    g_num: int = 4
    in_feats: int = 256
    h_feats: int = 128          # table row width (must be 128)
    n_classes: int = 40
    n_cores: int = 8
    win: int = 128              # dst nodes per SpMM window
    win_batch: int = 7          # windows per merged dma_gather call

    @property
    def shard(self):
        return self.n_nodes // self.n_cores

    @property
    def shard_p(self):          # padded shard rows
        return ((self.shard + P - 1) // P) * P

    @property
    def rows(self):             # padded table rows
        return self.shard_p * self.n_cores

    @property
    def half(self):
        return self.rows // 2

    @property
    def nwin(self):
        return self.shard_p // self.win

    @property
    def nbatch(self):
        return (self.nwin + self.win_batch - 1) // self.win_batch

    @property
    def cat(self):
        return self.h_feats * self.g_num

    @property
    def kc_cat(self):           # 128-chunks in cat dim
        return self.cat // P

    @property
    def kc_in(self):
        return self.in_feats // P

    @property
    def ntile(self):            # node tiles (128) in full padded table
        return self.rows // P


def _prep_inputs(cfg: Cfg, in_feat, src, dst, w, W1, W2, l1w, l1b, l2w, l2b,
                 l3w, l3b):
    """Host-side sharding/packing. Returns (in_maps, K_LO, K_HI)."""
    N, G = cfg.n_nodes, cfg.g_num
    SH, SHP = cfg.shard, cfg.shard_p
    NW, WIN, WB = cfg.nwin, cfg.win, cfg.win_batch
    HALF = cfg.half
    src = np.asarray(src).astype(np.int64)
    dst = np.asarray(dst).astype(np.int64)
    w = np.asarray(w, dtype=np.float32)
    in_feat = np.asarray(in_feat, dtype=np.float32)

    deg_out = np.empty((G, N), np.float32)
    deg_in = np.empty((G, N), np.float32)
    for g in range(G):
        deg_out[g] = np.clip(np.bincount(src[g], minlength=N), 1.0, None) ** -0.5
        deg_in[g] = np.clip(np.bincount(dst[g], minlength=N), 1.0, None) ** -0.5

    src_pad = (src // SH) * SHP + (src % SH)          # padded table row
    half_flag = (src_pad >= HALF).astype(np.int64)
    idx_local = (src_pad - half_flag * HALF).astype(np.int64)

    core_of = dst // SH
    dst_loc = dst % SH
    win_of = dst_loc // WIN
    dst_in_win = (dst_loc % WIN).astype(np.float32)

    # first pass: counts to fix K_LO / K_HI globally (one SPMD NEFF)
    maxlo = maxhi = 1
    buckets = {}
    for i in range(cfg.n_cores):
        for g in range(G):
            m = core_of[g] == i
            key = win_of[g][m] * 2 + half_flag[g][m]
            cnt = np.bincount(key, minlength=NW * 2)
            maxlo = max(maxlo, int(cnt[0::2].max()))
            maxhi = max(maxhi, int(cnt[1::2].max()))
            buckets[(i, g)] = m
    K_LO = (maxlo + P - 1) // P
    K_HI = (maxhi + P - 1) // P

    # all normalization folded into the per-edge selection weight
    w_eff = np.empty((G, src.shape[1]), np.float32)
    for g in range(G):
        w_eff[g] = w[g] * deg_in[g][dst[g]] * deg_out[g][src[g]]

    # replicated phase-1 inputs (identical for every core): full x, packed
    # transposed per tile for lhsT use
    xpad = np.zeros((cfg.rows, cfg.in_feats), np.float32)
    for i in range(cfg.n_cores):
        xpad[i * SHP:i * SHP + SH] = in_feat[i * SH:(i + 1) * SH]
    xt4 = xpad.reshape(cfg.ntile, P, cfg.kc_in, P)     # (t, n, kc, k)
    xtiles = np.ascontiguousarray(xt4.transpose(3, 0, 2, 1)).reshape(
        P, cfg.ntile, cfg.kc_in * P).astype(BF16)

    def pack_lhsT(W, kc):
        Wr = np.asarray(W, np.float32).reshape(kc, P, -1)   # (kc, k, fout)
        return np.ascontiguousarray(Wr.transpose(1, 0, 2)).reshape(P, -1)

    W1c = pack_lhsT(W1, cfg.kc_in).astype(BF16)
    W2c = pack_lhsT(W2, cfg.kc_cat).astype(BF16)
    l1wc = pack_lhsT(l1w, cfg.kc_cat).astype(BF16)
    l2wc = pack_lhsT(l2w, cfg.kc_cat).astype(BF16)
    l3wc = pack_lhsT(l3w, cfg.kc_cat).astype(BF16)
    l1bc = np.ascontiguousarray(
        np.asarray(l1b, np.float32).reshape(cfg.kc_cat, P).T)      # [128, kc]
    l2bc = np.ascontiguousarray(
        np.asarray(l2b, np.float32).reshape(cfg.kc_cat, P).T)
    l3bb = np.tile(np.asarray(l3b, np.float32)[None, :], (P, 1))   # [128, C]

    # repeated-iota constant for 2x-mode one-hot build:
    # iotaF[p, x*KBMAX + j] = x
    KBMAX = WB * max(K_LO, K_HI)
    if ONEHOT_2X:
        iotaF = np.repeat(np.arange(WIN, dtype=np.float32), KBMAX)[None, :]
        iotaF = np.ascontiguousarray(np.tile(iotaF, (P, 1))).astype(BF16)
    else:
        iotaF = np.zeros((P, WIN * KBMAX), np.float32)
        iotaF[:, :WIN] = np.arange(WIN, dtype=np.float32)[None, :]
        iotaF = iotaF.astype(BF16)

    NB = cfg.nbatch
    in_maps = []
    for i in range(cfg.n_cores):
        idx16 = {0: np.zeros((G, NW * K_LO * P), np.int16),
                 1: np.zeros((G, NW * K_HI * P), np.int16)}
        mdst = {0: np.zeros((G, P, NW * K_LO), np.float32),
                1: np.zeros((G, P, NW * K_HI), np.float32)}
        mw = {0: np.zeros((G, P, NW * K_LO), np.float32),
              1: np.zeros((G, P, NW * K_HI), np.float32)}
        for g in range(G):
            m = buckets[(i, g)]
            key = win_of[g][m] * 2 + half_flag[g][m]
            order = np.argsort(key, kind="stable")
            skey = key[order]
            cnt = np.bincount(skey, minlength=NW * 2)
            starts = np.concatenate([[0], np.cumsum(cnt)[:-1]])
            slot = np.arange(len(skey)) - starts[skey]
            il = idx_local[g][m][order]
            dw = dst_in_win[g][m][order]
            we = w_eff[g][m][order]
            swin = skey // 2
            shf = skey % 2
            for h, K in ((0, K_LO), (1, K_HI)):
                sel = shf == h
                pos = swin[sel] * (K * P) + slot[sel]     # (win, c, p) flat
                idx16[h][g][pos] = il[sel].astype(np.int16)
                c = slot[sel] // P
                p = slot[sel] % P
                mdst[h][g][p, swin[sel] * K + c] = dw[sel]
                mw[h][g][p, swin[sel] * K + c] = we[sel]

        def wrap(arr, K):
            # one contiguous idx stripe per (g, batch): [G, NB, 128, WB*K*8]
            out = np.zeros((G, NB, P, WB * K * 8), np.int16)
            for g in range(G):
                for b in range(NB):
                    w0 = b * WB
                    w1 = min(NW, w0 + WB)
                    fl = arr[g][w0 * K * P: w1 * K * P]
                    wr = fl.reshape(-1, 16).T          # [16, n]
                    out[g, b][:, :(w1 - w0) * K * 8] = np.tile(wr, (8, 1))
            return out

        im = {
            "xtiles": xtiles, "iotaF": iotaF,
            "w1c": W1c, "w2c": W2c, "l1wc": l1wc, "l2wc": l2wc,
            "l3wc": l3wc, "l1bc": l1bc, "l2bc": l2bc, "l3bb": l3bb,
            "idx_lo": wrap(idx16[0], K_LO), "idx_hi": wrap(idx16[1], K_HI),
            "mdst_lo": mdst[0].astype(BF16), "mdst_hi": mdst[1].astype(BF16),
            "mw_lo": mw[0].astype(BF16), "mw_hi": mw[1].astype(BF16),
        }
        in_maps.append(im)
    return in_maps, K_LO, K_HI


def _build(cfg: Cfg, K_LO, K_HI):
    G, NW, WIN, WB = cfg.g_num, cfg.nwin, cfg.win, cfg.win_batch
    NB = cfg.nbatch
    KC = cfg.kc_cat
    HF = cfg.h_feats
    CLS = cfg.n_classes
    KBMAX = WB * max(K_LO, K_HI)
    f32, bf16, i16, i32 = (mybir.dt.float32, mybir.dt.bfloat16,
                           mybir.dt.int16, mybir.dt.int32)

    nc = bacc.Bacc(num_swdge_queues=4)
    t_xt = nc.declare_dram_parameter("xtiles", [P, cfg.ntile, cfg.kc_in * P], bf16, isOutput=False)
    t_iota = nc.declare_dram_parameter("iotaF", [P, WIN * KBMAX], bf16, isOutput=False)
    t_w1 = nc.declare_dram_parameter("w1c", [P, cfg.kc_in * HF], bf16, isOutput=False)
    t_w2 = nc.declare_dram_parameter("w2c", [P, KC * HF], bf16, isOutput=False)
    t_l1w = nc.declare_dram_parameter("l1wc", [P, KC * cfg.cat], bf16, isOutput=False)
    t_l2w = nc.declare_dram_parameter("l2wc", [P, KC * cfg.cat], bf16, isOutput=False)
    t_l3w = nc.declare_dram_parameter("l3wc", [P, KC * CLS], bf16, isOutput=False)
    t_l1b = nc.declare_dram_parameter("l1bc", [P, KC], f32, isOutput=False)
    t_l2b = nc.declare_dram_parameter("l2bc", [P, KC], f32, isOutput=False)
    t_l3b = nc.declare_dram_parameter("l3bb", [P, CLS], f32, isOutput=False)
    t_ilo = nc.declare_dram_parameter("idx_lo", [G, NB, P, WB * K_LO * 8], i16, isOutput=False)
    t_ihi = nc.declare_dram_parameter("idx_hi", [G, NB, P, WB * K_HI * 8], i16, isOutput=False)
    t_mdl = nc.declare_dram_parameter("mdst_lo", [G, P, NW * K_LO], bf16, isOutput=False)
    t_mdh = nc.declare_dram_parameter("mdst_hi", [G, P, NW * K_HI], bf16, isOutput=False)
    t_mwl = nc.declare_dram_parameter("mw_lo", [G, P, NW * K_LO], bf16, isOutput=False)
    t_mwh = nc.declare_dram_parameter("mw_hi", [G, P, NW * K_HI], bf16, isOutput=False)
    t_out = nc.declare_dram_parameter("out", [WIN, NW * CLS], f32, isOutput=True)

    d_t1 = nc.dram_tensor("t1", [cfg.rows, HF], bf16)
    d_t2s = nc.dram_tensor("t2s", [cfg.shard_p, HF], bf16)
    d_t2f = nc.dram_tensor("t2f", [cfg.rows, HF], bf16, addr_space="Shared")

    AF = mybir.ActivationFunctionType
    qctr = [0]

    with tile.TileContext(nc) as tc:
        with (
            tc.tile_pool(name="const", bufs=1) as cp,
            tc.tile_pool(name="x", bufs=3) as xp,
            tc.tile_pool(name="gath", bufs=2) as gp,
            tc.tile_pool(name="hcat", bufs=2) as hp,
            tc.tile_pool(name="dense", bufs=3) as dp,
            tc.tile_pool(name="psa", bufs=2, space="PSUM") as pm,
            tc.tile_pool(name="psb", bufs=2, space="PSUM") as pb,
        ):
            # constants
            ident = cp.tile([P, P], f32)
            make_identity(nc, ident[:])

            def const_load(t, shape, dtype):
                s = cp.tile(shape, dtype, tag=t.name + "_c")
                nc.sync.dma_start(out=s[:], in_=t[:])
                return s

            iota_sb = const_load(t_iota, [P, WIN * KBMAX], bf16)
            w1_sb = const_load(t_w1, [P, cfg.kc_in * HF], bf16)
            w2_sb = const_load(t_w2, [P, KC * HF], bf16)
            l1w_sb = const_load(t_l1w, [P, KC * cfg.cat], bf16)
            l2w_sb = const_load(t_l2w, [P, KC * cfg.cat], bf16)
            l3w_sb = const_load(t_l3w, [P, KC * CLS], bf16)
            l1b_sb = const_load(t_l1b, [P, KC], f32)
            l2b_sb = const_load(t_l2b, [P, KC], f32)
            l3b_sb = const_load(t_l3b, [P, CLS], f32)
            out_sb = cp.tile([WIN, NW * CLS], f32)

            # ---------------- phase 1: replicated T1 = x @ W1 ----------------
            # batched tile-group DMAs keep phase 1 off the sync-queue
            # dispatch-rate limit; psum rotates across the two idle pools
            TG = 8                       # tiles per DMA group
            KIP = cfg.kc_in * P
            for t0 in range(0, cfg.ntile, TG):
                tg = min(TG, cfg.ntile - t0)
                xt = xp.tile([P, TG * KIP], bf16, tag="xt")
                nc.sync.dma_start(
                    out=xt[:, :tg * KIP].rearrange("p (t f) -> p t f", f=KIP),
                    in_=t_xt[:, t0:t0 + tg, :])
                h1row = xp.tile([P, TG * HF], bf16, tag="h1row")
                for ti in range(tg):
                    pool = pb if ti % 2 == 0 else pm
                    ptag = "misc" if ti % 2 == 0 else "mlp"
                    q1 = pool.tile([P, HF], f32, tag=ptag)
                    xo = ti * KIP
                    for kc in range(cfg.kc_in):
                        nc.tensor.matmul(
                            out=q1[:], lhsT=xt[:, xo + kc * P:xo + (kc + 1) * P],
                            rhs=w1_sb[:, kc * HF:(kc + 1) * HF],
                            start=(kc == 0), stop=(kc == cfg.kc_in - 1))
                    if ti % 2 == 0:
                        nc.scalar.activation(
                            h1row[:, ti * HF:(ti + 1) * HF], q1[:], AF.Copy)
                    else:
                        nc.vector.tensor_copy(
                            h1row[:, ti * HF:(ti + 1) * HF], q1[:])
                # store rows (t, p, f) from sbuf layout (p, t, f)
                h1a = h1row[:]
                t1a = d_t1[:]
                out_ap = bass.AP(t1a.tensor, t1a.offset + t0 * P * HF,
                                 [[HF, P], [P * HF, tg], [1, HF]])
                in_ap = bass.AP(h1a.tensor, h1a.offset,
                                [list(h1a.ap[0]), [HF, tg], [1, HF]])
                nc.sync.dma_start(out=out_ap, in_=in_ap)

            tc.strict_bb_all_engine_barrier()

            # ------------- SpMM + dense layers, per window batch -------------
            def spmm_layer(table, layer2):
                for b in range(NB):
                    w0 = b * WB
                    w1 = min(NW, w0 + WB)
                    nw = w1 - w0
                    hcat_t = {}
                    for g in range(G):
                        feats = {}
                        sels = {}
                        kbs = {}
                        for h, K, t_i, t_md, t_mw in (
                            (0, K_LO, t_ilo, t_mdl, t_mwl),
                            (1, K_HI, t_ihi, t_mdh, t_mwh),
                        ):
                            KB = nw * K
                            kbs[h] = KB
                            idx_t = gp.tile([P, WB * K * 8], i16, tag=f"idx{h}")
                            nc.sync.dma_start(
                                out=idx_t[:, :KB * 8],
                                in_=t_i[g, b][:, :KB * 8])
                            md_t = gp.tile([P, WB * K], bf16, tag=f"md{h}")
                            nc.sync.dma_start(out=md_t[:, :KB],
                                              in_=t_md[g][:, w0 * K:w1 * K])
                            mw_t = gp.tile([P, WB * K], bf16, tag=f"mw{h}")
                            nc.sync.dma_start(out=mw_t[:, :KB],
                                              in_=t_mw[g][:, w0 * K:w1 * K])
                            ft = gp.tile([P, WB * K * HF], bf16, tag=f"ft{h}")
                            GCH = 8        # chunks (of 128 idx) per gather call (HW ring limit)
                            for j in range(0, KB, GCH):
                                gl = min(GCH, KB - j)
                                nc.gpsimd.dma_gather(
                                    out_ap=ft[:, j * HF:(j + gl) * HF].rearrange(
                                        "p (k f) -> p k f", f=HF),
                                    in_ap=table[(cfg.half if h else 0):
                                                (cfg.rows if h else cfg.half), :],
                                    idxs_ap=idx_t[:, j * 8:(j + gl) * 8],
                                    num_idxs=gl * P, num_idxs_reg=gl * P,
                                    elem_size=HF, elem_step=HF,
                                    queue_num=qctr[0] % 4,
                                )
                                qctr[0] += 1
                            st = gp.tile([P, WIN * WB * K], bf16, tag=f"st{h}")
                            if ONEHOT_2X:
                                # st[p, x, j] = (md[p,j] == x) * mw[p,j]
                                st3 = st[:, :WIN * KB].rearrange(
                                    "p (x k) -> p x k", k=KB)
                                mda = md_t[:]
                                md_b = bass.AP(mda.tensor, mda.offset,
                                               [list(mda.ap[0]), [0, WIN],
                                                [1, KB]])
                                ioa = iota_sb[:]
                                io_b = bass.AP(ioa.tensor, ioa.offset,
                                               [list(ioa.ap[0]), [KBMAX, WIN],
                                                [1, KB]])
                                nc.vector.tensor_tensor(
                                    out=st3, in0=md_b, in1=io_b,
                                    op=mybir.AluOpType.is_equal)
                                mwa = mw_t[:]
                                mw_b = bass.AP(mwa.tensor, mwa.offset,
                                               [list(mwa.ap[0]), [0, WIN],
                                                [1, KB]])
                                nc.vector.tensor_tensor(
                                    out=st3, in0=st3, in1=mw_b,
                                    op=mybir.AluOpType.mult)
                            else:
                                # baseline layout: st[p, j, x]
                                st3 = st[:, :KB * WIN].rearrange(
                                    "p (k x) -> p k x", x=WIN)
                                ioa = iota_sb[:]
                                io_b = bass.AP(ioa.tensor, ioa.offset,
                                               [list(ioa.ap[0]), [0, KB],
                                                [1, WIN]])
                                nc.vector.tensor_tensor(
                                    out=st3,
                                    in0=md_t[:, :KB].to_broadcast(
                                        [P, KB, WIN]),
                                    in1=io_b,
                                    op=mybir.AluOpType.is_equal)
                                nc.vector.tensor_tensor(
                                    out=st3, in0=st3,
                                    in1=mw_t[:, :KB].to_broadcast(
                                        [P, KB, WIN]),
                                    op=mybir.AluOpType.mult)
                            feats[h] = ft
                            sels[h] = st
                        for wi in range(w0, w1):
                            dw = wi - w0
                            ps = pm.tile([P, WIN], f32, tag="agg")
                            tot = K_LO + K_HI
                            ci = 0
                            for h, K in ((0, K_LO), (1, K_HI)):
                                ft, st = feats[h], sels[h]
                                KB = kbs[h]
                                sta = st[:]
                                for c in range(K):
                                    cc = dw * K + c
                                    if ONEHOT_2X:
                                        rhs = bass.AP(
                                            sta.tensor, sta.offset + cc,
                                            [list(sta.ap[0]), [KB, WIN]])
                                    else:
                                        rhs = st[:, cc * WIN:(cc + 1) * WIN]
                                    nc.tensor.matmul(
                                        out=ps[:],
                                        lhsT=ft[:, cc * HF:(cc + 1) * HF],
                                        rhs=rhs,
                                        start=(ci == 0), stop=(ci == tot - 1))
                                    ci += 1
                            hc = hp.tile([P, WIN], bf16, tag=f"hc{dw}_{g}")
                            nc.scalar.activation(hc[:], ps[:], AF.Relu)
                            hcat_t[(wi, g)] = hc
                    for wi in range(w0, w1):
                        hcat = [hcat_t[(wi, g)] for g in range(G)]
                        if not layer2:
                            def mlp(ws, bs, ins, name):
                                outs = []
                                for fc in range(KC):
                                    ps = pm.tile([P, WIN], f32, tag="mlp")
                                    for kc in range(KC):
                                        nc.tensor.matmul(
                                            out=ps[:],
                                            lhsT=ws[:, (kc * KC + fc) * P:
                                                    (kc * KC + fc + 1) * P],
                                            rhs=ins[kc][:],
                                            start=(kc == 0), stop=(kc == KC - 1))
                                    o = dp.tile([P, WIN], bf16,
                                                tag=f"mlpo{name}{fc}")
                                    nc.scalar.activation(o[:], ps[:], AF.Relu,
                                                         bias=bs[:, fc:fc + 1])
                                    outs.append(o)
                                return outs
                            hl1 = mlp(l1w_sb, l1b_sb, hcat, "a")
                            hl2 = mlp(l2w_sb, l2b_sb, hl1, "b")
                            p2 = pb.tile([P, WIN], f32, tag="misc")
                            for kc in range(KC):
                                nc.tensor.matmul(
                                    out=p2[:],
                                    lhsT=w2_sb[:, kc * HF:(kc + 1) * HF],
                                    rhs=hl2[kc][:],
                                    start=(kc == 0), stop=(kc == KC - 1))
                            p2s = dp.tile([P, WIN], f32, tag="p2s")
                            nc.scalar.activation(p2s[:], p2[:], AF.Copy)
                            p2t = pb.tile([WIN, P], f32, tag="misc")
                            nc.tensor.transpose(p2t[:], p2s[:], ident[:])
                            h2r = dp.tile([WIN, HF], bf16, tag="h2r")
                            nc.scalar.activation(h2r[:], p2t[:], AF.Copy)
                            nc.sync.dma_start(
                                out=d_t2s[wi * WIN:(wi + 1) * WIN, :], in_=h2r[:])
                        else:
                            ps = pb.tile([WIN, CLS], f32, tag="misc")
                            for kc in range(KC):
                                nc.tensor.matmul(
                                    out=ps[:],
                                    lhsT=hcat[kc][:],
                                    rhs=l3w_sb[:, kc * CLS:(kc + 1) * CLS],
                                    start=(kc == 0), stop=(kc == KC - 1))
                            nc.vector.tensor_tensor(
                                out=out_sb[:, wi * CLS:(wi + 1) * CLS],
                                in0=ps[:], in1=l3b_sb[:WIN, :],
                                op=mybir.AluOpType.add)

            spmm_layer(d_t1, layer2=False)

            tc.strict_bb_all_engine_barrier()
            nc.gpsimd.collective_compute(
                "AllGather", mybir.AluOpType.bypass,
                ins=[d_t2s[:]], outs=[d_t2f[:]],
                replica_groups=[list(range(cfg.n_cores))],
            )
            tc.strict_bb_all_engine_barrier()

            spmm_layer(d_t2f, layer2=True)

            nc.sync.dma_start(out=t_out[:], in_=out_sb[:])
    nc.finalize()
    return nc


def _run(cfg: Cfg, inputs: dict, trace: bool = False):
    _install_ntff_hook()
    from concourse import bass_utils
    bass_utils.upload_artifacts = lambda d: "local://skipped"
    from concourse.bass_utils import run_bass_kernel_spmd

    in_maps, K_LO, K_HI = _prep_inputs(cfg, **inputs)
    nc = _build(cfg, K_LO, K_HI)
    res = run_bass_kernel_spmd(nc, in_maps, list(range(cfg.n_cores)),
                               trace=trace)
    outs = []
    for i in range(cfg.n_cores):
        o = res.results[i]["out"]                     # [WIN, nwin*CLS]
        o = o.reshape(cfg.win, cfg.nwin, cfg.n_classes).transpose(1, 0, 2)
        outs.append(o.reshape(cfg.shard_p, cfg.n_classes)[:cfg.shard])
    full = np.concatenate(outs, axis=0)
    return full, res.exec_time_ns


def kernel(**inputs) -> np.ndarray:
    cfg = Cfg()
    out, _ = _run(cfg, inputs, trace=False)
    return out.astype(np.float32)
